# revision 41
# baseline (speedup 1.0000x reference)
"""Causal linear attention (elu+1 feature map) Trainium2 Bass kernel.

Full inputs q,k,v: [4, 2048, 12, 64] fp32 -> out [4, 2048, 12, 64] fp32.
Sharding: 48 (batch, head) pairs, 6 per core across 8 NeuronCores.

bf16 end-to-end on device (inputs cast on host, output cast back):
matmuls run at 1 cycle/row, DMA bytes halve, DVE gets 2x/4x modes.

Per (n,h) pair, per 128-chunk g (exact algebraic regrouping):
  qf = phi(q), kf = phi(k), phi(x) = exp(min(x,0)) + max(x,0)
  state S_g = sum_{g'<g} kfc_{g'}^T @ [v | 1]   (PSUM kv outer products +
              one segmented DVE scan straight off PSUM)
  scoresT[s,c] = sum_d kfT[d,s] qfT[d,c], masked to s<=c
  num' = qfc @ S_{g-1} + scoresT-contract @ [v | 1]  -> [128, 65]
  out = num'[:, :64] * (1 / num'[:, 64])  (DVE reciprocal off the PSUM z
        column + one broadcast multiply per 4-chunk group)

Pairs are processed two at a time stacked on partition halves. The
kn layout ([s, g, d] with chunk position on partitions, needed as the
kv-matmul stationary operand) is produced from the phi'd kT tile by a
single XBAR DMA transpose per pair-pair: transposing [128(pair,d),
2048(g,s)] into [128(s), 16(g), 128(pair,d)] transposes each 128x128
chunk in place, so both pairs' kn tiles come out stacked correctly.
"""

import json
import os

import numpy as np

# ---------------------------------------------------------------------------
# Workaround for walrus "Too many sync wait commands": cap waits per
# instruction at 1, hoisting overflow onto same-engine NoOps inserted
# immediately before (engines run their stream in order, so consecutive
# waits AND together identically).
# ---------------------------------------------------------------------------
_wsplit_counter = [0]


def _split_instruction_waits(inst):
    si = inst.get("sync_info")
    if not si:
        return []
    waits = si.get("on_wait") or []
    if len(waits) <= 1:
        return []
    si["on_wait"] = waits[-1:]
    nops = []
    for w in waits[:-1]:
        _wsplit_counter[0] += 1
        nops.append(
            {
                "debug": inst.get("debug", 0),
                "engine": inst["engine"],
                "ins": [],
                "name": f"I-wsplit-{_wsplit_counter[0]}",
                "opcode": "NoOp",
                "outs": [],
                "sync_info": {"on_update": [], "on_wait": [w]},
            }
        )
    return nops


def _fix_module_json(raw: bytes) -> bytes:
    m = json.loads(raw)
    changed = False
    for f in m.get("functions", []):
        for b in f.get("blocks", []):
            out = []
            for inst in b.get("instructions", []):
                nops = _split_instruction_waits(inst)
                if nops:
                    changed = True
                    out.extend(nops)
                out.append(inst)
            b["instructions"] = out
    return json.dumps(m).encode() if changed else raw


_patch_installed = [False]


def _install_bir_patch():
    if _patch_installed[0]:
        return
    _patch_installed[0] = True
    import concourse.bass as _bass

    _orig = _bass.Bass.to_json_bytes

    def _patched(self):
        return _fix_module_json(_orig(self))

    _bass.Bass.to_json_bytes = _patched


# ---------------------------------------------------------------------------
# Problem constants (hardcoded per contest contract)
# ---------------------------------------------------------------------------
B, L, H, D = 4, 2048, 12, 64
CHUNK = 128
G = L // CHUNK  # 16
N_CORES = 8
PAIRS = [(n, h) for n in range(B) for h in range(H)]  # 48
PER_CORE = len(PAIRS) // N_CORES  # 6
NPP = PER_CORE // 2  # pair-pairs per core


DEFAULT_CFG = {
    "tpos": "after_q",   # transpose emission: after_k | after_q | mixed
    "qphi": "split",     # pool_mf | pool_il | dve | split (Pool+DVE)
    "kphi": "split",     # dve | split
    "sc_route": "appapapp",  # per group: d=dve, a=actdve, p=actpool
    "nm_route": "dddddddd",  # per group: d=dve, a=act-chunks
    "bufs": (3, 3, 3),
    "loads": "half",
    "warmup": 0,
}


def _build_bass(reps: int = 1, cfg: dict | None = None):
    cfg = {**DEFAULT_CFG, **(cfg or {})}
    import concourse.bass as bass
    import concourse.tile as tile
    import concourse.mybir as mybir

    fp32 = mybir.dt.float32
    bf16 = mybir.dt.bfloat16
    AF = mybir.ActivationFunctionType
    ALU = mybir.AluOpType

    nc = bass.Bass()
    qt = nc.dram_tensor("qt", [NPP, 2 * D, L], bf16, kind="ExternalInput")
    kt = nc.dram_tensor("kt", [NPP, 2 * D, L], bf16, kind="ExternalInput")
    # v with the ones column baked in host-side: contiguous 2080B rows
    vn = nc.dram_tensor("vn", [PER_CORE, CHUNK, G, D + 1], bf16, kind="ExternalInput")
    mask = nc.dram_tensor("mask", [CHUNK, CHUNK], bf16, kind="ExternalInput")
    on = nc.dram_tensor("on", [PER_CORE, CHUNK, G, D], bf16, kind="ExternalOutput")

    SCB = 4  # chunks per PSUM bank batch

    with tile.TileContext(nc) as tc:
        with (
            tc.tile_pool(name="singles", bufs=1) as singles,
            tc.tile_pool(name="ins", bufs=cfg["bufs"][0]) as ins,
            tc.tile_pool(name="work", bufs=cfg["bufs"][1]) as work,
            tc.tile_pool(name="outs", bufs=cfg["bufs"][2]) as outs,
            tc.tile_pool(name="ps_sc", bufs=3, space="PSUM") as ps_sc,
            tc.tile_pool(name="ps_num", bufs=2, space="PSUM") as ps_num,
            tc.tile_pool(name="ps_kv", bufs=1, space="PSUM") as ps_kv_pool,
            tc.tile_pool(name="ps_kz", bufs=1, space="PSUM") as ps_kz_pool,
        ):
            maskbuf = singles.tile([CHUNK, SCB, CHUNK], bf16)
            segreset = singles.tile([CHUNK, D + 1, G], bf16)

            def emit_singles():
                # on Pool so DVE is free for phi from the first microsecond
                nc.sync.dma_start(out=maskbuf[:, 0, :], in_=mask[:])
                for i in range(1, SCB):
                    nc.gpsimd.tensor_copy(out=maskbuf[:, i, :], in_=maskbuf[:, 0, :])
                # segmented-scan reset pattern (g innermost): 0 at g==0, 1 else
                nc.gpsimd.memset(segreset[:], 1.0)
                nc.gpsimd.memset(segreset[:, :, 0:1], 0.0)

            # pp-level software pipeline: emission order per engine is
            # program order, so phi for pp j+1 is emitted BEFORE the
            # scores/num pipeline of pp j (DVE/Pool/ACT start it while PE
            # still works on pp j), and out DMAs are emitted after the next
            # pp's input loads so SP's in-order issue never starves.
            pending_outs = []

            def emit_loads(j):
                hp = tc.high_priority()
                hp.__enter__()
                qT2f = ins.tile([2 * D, L], bf16, tag="qT2f")
                kT2f = ins.tile([2 * D, L], bf16, tag="kT2f")
                vt2 = ins.tile([CHUNK, 2, G, D + 1], bf16, tag="vt2")
                if cfg["loads"] == "half":
                    for h in (0, 1):
                        hs = slice(h * (L // 2), (h + 1) * (L // 2))
                        nc.sync.dma_start(out=kT2f[:, hs], in_=kt[j, :, hs])
                        nc.sync.dma_start(out=qT2f[:, hs], in_=qt[j, :, hs])
                else:
                    nc.sync.dma_start(out=kT2f[:], in_=kt[j])
                    nc.sync.dma_start(out=qT2f[:], in_=qt[j])
                for s in (0, 1):
                    nc.sync.dma_start(out=vt2[:, s, :, :], in_=vn[2 * j + s])
                hp.__exit__(None, None, None)
                return qT2f, kT2f, vt2

            def flush_outs():
                for out_ap, in_ap in pending_outs:
                    nc.sync.dma_start(out=out_ap, in_=in_ap)
                pending_outs.clear()

            def emit_phi(j, tiles, tpos=None):
                qT2f, kT2f, vt2 = tiles
                hp = tc.high_priority()
                hp.__enter__()
                qT2 = work.tile([2 * D, L], bf16, tag="qT2")
                kT2 = work.tile([2 * D, L], bf16, tag="kT2")
                mx = work.tile([2 * D, L], bf16, tag="mx")
                mxq = work.tile([2 * D, L], bf16, tag="mxq")
                kn2b = work.tile([CHUNK, G, 2 * D], bf16, tag="kn2b")
                kmax = {0: nc.vector, 1: nc.vector}
                if cfg.get("kphi") == "split":
                    kmax = {0: nc.gpsimd, 1: nc.vector}
                for h in (0, 1):
                    hs = slice(h * (L // 2), (h + 1) * (L // 2))
                    nc.vector.tensor_scalar_min(
                        out=kT2[:, hs], in0=kT2f[:, hs], scalar1=0.0
                    )
                    nc.scalar.activation(out=kT2[:, hs], in_=kT2[:, hs], func=AF.Exp)
                    kmax[h].tensor_scalar_max(
                        out=mx[:, hs], in0=kT2f[:, hs], scalar1=0.0
                    )
                    nc.vector.tensor_tensor(
                        out=kT2[:, hs], in0=kT2[:, hs], in1=mx[:, hs], op=ALU.add
                    )
                tpos = tpos or cfg["tpos"]
                if tpos == "after_k":
                    nc.scalar.dma_start(out=kn2b[:], in_=kT2[:], transpose=True)
                qeng = nc.vector if cfg["qphi"] == "dve" else nc.gpsimd
                if cfg["qphi"] in ("pool_mf", "split"):
                    qmin = {0: qeng, 1: qeng}
                    qmax = {0: qeng, 1: qeng}
                    if cfg["qphi"] == "split":
                        qmin = {0: nc.gpsimd, 1: nc.vector}
                        qmax = {0: nc.vector, 1: nc.gpsimd}
                    for h in (0, 1):
                        hs = slice(h * (L // 2), (h + 1) * (L // 2))
                        qmin[h].tensor_scalar_min(
                            out=qT2[:, hs], in0=qT2f[:, hs], scalar1=0.0
                        )
                        nc.scalar.activation(
                            out=qT2[:, hs], in_=qT2[:, hs], func=AF.Exp
                        )
                    for h in (0, 1):
                        hs = slice(h * (L // 2), (h + 1) * (L // 2))
                        qmax[h].tensor_scalar_max(
                            out=mxq[:, hs], in0=qT2f[:, hs], scalar1=0.0
                        )
                        nc.vector.tensor_tensor(
                            out=qT2[:, hs], in0=qT2[:, hs], in1=mxq[:, hs],
                            op=ALU.add,
                        )
                else:
                    for h in (0, 1):
                        hs = slice(h * (L // 2), (h + 1) * (L // 2))
                        qeng.tensor_scalar_min(
                            out=qT2[:, hs], in0=qT2f[:, hs], scalar1=0.0
                        )
                        nc.scalar.activation(
                            out=qT2[:, hs], in_=qT2[:, hs], func=AF.Exp
                        )
                        qeng.tensor_scalar_max(
                            out=mxq[:, hs], in0=qT2f[:, hs], scalar1=0.0
                        )
                        nc.vector.tensor_tensor(
                            out=qT2[:, hs], in0=qT2[:, hs], in1=mxq[:, hs],
                            op=ALU.add,
                        )
                if tpos == "after_q":
                    nc.scalar.dma_start(out=kn2b[:], in_=kT2[:], transpose=True)
                hp.__exit__(None, None, None)
                return qT2, kT2, vt2, kn2b

            def emit_state(j, phi_tiles):
                qT2, kT2, vt2, kn2b = phi_tiles
                hp = tc.high_priority()
                hp.__enter__()
                # per-chunk kv outer products, both pairs stacked; z column
                # in its own small PSUM tile so kv fits 2 banks
                kv_ps = ps_kv_pool.tile([CHUNK, G, D], fp32, tag="kv")
                kz_ps = ps_kz_pool.tile([CHUNK, G], fp32, tag="kz")
                for s in (0, 1):
                    po = D * s
                    for g in range(G):
                        nc.tensor.matmul(
                            kv_ps[po : po + D, g, :],
                            kn2b[:, g, po : po + D],
                            vt2[:, s, g, 0:D],
                            start=True, stop=True, skip_group_check=True,
                        )
                        nc.tensor.matmul(
                            kz_ps[po : po + D, g : g + 1],
                            kn2b[:, g, po : po + D],
                            vt2[:, s, g, D : D + 1],
                            start=True, stop=True, skip_group_check=True,
                        )
                kv_all = work.tile([CHUNK, D + 1, G], bf16, tag="kv_all")
                if cfg.get("kvcopy") == "split":
                    nc.scalar.copy(
                        out=kv_all[:, 0 : D // 2, :],
                        in_=kv_ps[:, :, 0 : D // 2].rearrange("p g m -> p m g"),
                    )
                    nc.vector.tensor_copy(
                        out=kv_all[:, D // 2 : D, :],
                        in_=kv_ps[:, :, D // 2 : D].rearrange("p g m -> p m g"),
                    )
                else:
                    nc.scalar.copy(
                        out=kv_all[:, 0:D, :],
                        in_=kv_ps[:].rearrange("p g m -> p m g"),
                    )
                nc.scalar.copy(out=kv_all[:, D, :], in_=kz_ps[:])
                kv_scan = work.tile([CHUNK, D + 1, G], bf16, tag="kv_scan")
                nc.vector.tensor_tensor_scan(
                    out=kv_scan[:].rearrange("p m g -> p (m g)"),
                    data0=segreset[:].rearrange("p m g -> p (m g)"),
                    data1=kv_all[:].rearrange("p m g -> p (m g)"),
                    initial=0.0,
                    op0=ALU.mult,
                    op1=ALU.add,
                )
                hp.__exit__(None, None, None)
                return qT2, kT2, vt2, kv_scan

            _scr = cfg["sc_route"] * (24 // len(cfg["sc_route"]) + 1)
            _nmr = cfg["nm_route"] * (24 // len(cfg["nm_route"]) + 1)
            SC_ROUTE = [
                {"d": "dve", "a": "actdve", "p": "actpool"}[c] for c in _scr[:24]
            ]
            NM_ROUTE = [{"d": "dve", "a": "act"}[c] for c in _nmr[:24]]

            def make_group_emitters(j, st, goff):
                qT2, kT2, vt2, kv_scan = st
                out_stage = outs.tile([CHUNK, 2, G, D], bf16, tag="out_stage")
                groups = [(gb, s) for s in (0, 1) for gb in range(G // SCB)]
                state = {}

                def emit_sc(t):
                    gb, s = groups[t]
                    po = D * s
                    sc_ps = ps_sc.tile([CHUNK, SCB, CHUNK], fp32, tag="sc")
                    for i in range(SCB):
                        g = gb * SCB + i
                        cs = slice(g * CHUNK, (g + 1) * CHUNK)
                        nc.tensor.matmul(
                            sc_ps[:, i, :],
                            kT2[po : po + D, cs],
                            qT2[po : po + D, cs],
                            start=True, stop=True, skip_group_check=True,
                        )
                    state[t] = [sc_ps, None]

                def emit_evac_num(t):
                    gb, s = groups[t]
                    po = D * s
                    sc_ps = state[t][0]
                    sc_sb = work.tile([CHUNK, SCB, CHUNK], bf16, tag="sc_sb", bufs=10)
                    route = SC_ROUTE[goff + t]
                    if route == "dve":
                        nc.vector.tensor_tensor(
                            out=sc_sb[:], in0=sc_ps[:], in1=maskbuf[:], op=ALU.mult
                        )
                    else:
                        sc_raw = work.tile([CHUNK, SCB, CHUNK], bf16, tag="sc_raw", bufs=6)
                        nc.scalar.copy(out=sc_raw[:], in_=sc_ps[:])
                        meng = nc.vector if route == "actdve" else nc.gpsimd
                        meng.tensor_tensor(
                            out=sc_sb[:], in0=sc_raw[:], in1=maskbuf[:], op=ALU.mult
                        )
                    num_ps = ps_num.tile([CHUNK, SCB, D + 1], fp32, tag="num")
                    for i in range(SCB):
                        g = gb * SCB + i
                        cs = slice(g * CHUNK, (g + 1) * CHUNK)
                        if g > 0:
                            nc.tensor.matmul(
                                num_ps[:, i, :],
                                qT2[po : po + D, cs],
                                kv_scan[po : po + D, :, g - 1],
                                start=True, stop=False, skip_group_check=True,
                            )
                        nc.tensor.matmul(
                            num_ps[:, i, :],
                            sc_sb[:, i, :],
                            vt2[:, s, g, :],
                            start=(g == 0), stop=True, skip_group_check=True,
                        )
                    state[t][1] = num_ps

                def emit_norm(t):
                    gb, s = groups[t]
                    num_ps = state.pop(t)[1]
                    r4 = work.tile([CHUNK, SCB], fp32, tag="r4", bufs=6)
                    nc.vector.reciprocal(out=r4[:], in_=num_ps[:, :, D])
                    if NM_ROUTE[goff + t] == "dve":
                        nc.vector.tensor_tensor(
                            out=out_stage[:, s, gb * SCB : (gb + 1) * SCB, :],
                            in0=num_ps[:, :, 0:D],
                            in1=r4[:].broadcast_to([CHUNK, SCB, D]),
                            op=ALU.mult,
                        )
                    else:
                        for i in range(SCB):
                            g = gb * SCB + i
                            nc.scalar.mul(
                                out=out_stage[:, s, g, :],
                                in_=num_ps[:, i, 0:D],
                                mul=r4[:, i : i + 1],
                            )

                def finish(t):
                    # queue the finished gb slice (s=0 gb0..3 then s=1 gb0..3)
                    s_ = t // 4
                    gb_ = t % 4
                    gsl = slice(gb_ * SCB, (gb_ + 1) * SCB)
                    pending_outs.append(
                        (on[2 * j + s_][:, gsl, :], out_stage[:, s_, gsl, :])
                    )

                return emit_sc, emit_evac_num, emit_norm, finish

            # -- merged cross-pp pipeline: one continuous stream of groups
            # over all pair-pairs; sc matmuls run 2 groups ahead of the
            # evacuation, normalize trails one more. Next pp's phi/state
            # and loads are interleaved at fixed slots.
            pps = [jj for _ in range(reps) for jj in range(NPP)]
            NG = 2 * (G // SCB)  # groups per pp
            npp_total = len(pps)
            raw = {}
            phi = {}
            emitters = {}

            if cfg["warmup"]:
                with tc.high_priority():
                    wtile = singles.tile([CHUNK, CHUNK], bf16, name="warm_in")
                    nc.gpsimd.memset(wtile[:], 0.0)
                    warm_ps = ps_sc.tile([CHUNK, SCB, CHUNK], fp32, tag="sc")
                    for i in range(cfg["warmup"]):
                        nc.tensor.matmul(
                            warm_ps[:, i % SCB, :], wtile[:], wtile[:],
                            start=True, stop=True, skip_group_check=True,
                        )
            raw[0] = emit_loads(pps[0])
            emit_singles()
            tp0 = "after_k" if cfg["tpos"] == "mixed" else cfg["tpos"]
            tps = "after_q" if cfg["tpos"] == "mixed" else cfg["tpos"]
            phi[0] = emit_phi(pps[0], raw.pop(0), tpos=tp0)
            emitters[0] = make_group_emitters(
                pps[0], emit_state(pps[0], phi.pop(0)), 0
            )
            if npp_total > 1:
                raw[1] = emit_loads(pps[1])

            total_slots = npp_total * NG + 3
            for u in range(total_slots):
                idx, t = divmod(u, NG)
                if t == 1 and idx + 1 < npp_total:
                    phi[idx + 1] = emit_phi(pps[idx + 1], raw.pop(idx + 1), tpos=tps)
                if t == 1 and idx + 2 < npp_total:
                    raw[idx + 2] = emit_loads(pps[idx + 2])
                    flush_outs()
                if t == 5 and idx + 1 < npp_total:
                    emitters[idx + 1] = make_group_emitters(
                        pps[idx + 1],
                        emit_state(pps[idx + 1], phi.pop(idx + 1)),
                        (idx + 1) * NG,
                    )
                if u < npp_total * NG:
                    emitters[idx][0](t)  # sc
                if 2 <= u < npp_total * NG + 2:
                    i2, t2 = divmod(u - 2, NG)
                    emitters[i2][1](t2)  # evac + num
                if 3 <= u < npp_total * NG + 3:
                    i3, t3 = divmod(u - 3, NG)
                    emitters[i3][2](t3)  # norm
                    emitters[i3][3](t3)  # queue completed gb slice
                    if t3 == NG - 1:
                        del emitters[i3]
            flush_outs()
    return nc


_cached = {}


def _prep_inputs(q, k, v):
    import ml_dtypes

    bf = ml_dtypes.bfloat16
    maskarr = np.ascontiguousarray(
        np.tril(np.ones((CHUNK, CHUNK), np.float32)).T
    ).astype(bf)  # [s, c] : 1 if s<=c
    in_maps = []
    for c in range(N_CORES):
        sel = PAIRS[c * PER_CORE : (c + 1) * PER_CORE]
        qt = np.ascontiguousarray(
            np.stack([q[n, :, h, :].T for (n, h) in sel]).astype(bf)
        ).reshape(NPP, 2 * D, L)
        kt = np.ascontiguousarray(
            np.stack([k[n, :, h, :].T for (n, h) in sel]).astype(bf)
        ).reshape(NPP, 2 * D, L)
        vn = np.ascontiguousarray(
            np.concatenate(
                [
                    np.stack(
                        [
                            v[n, :, h, :].reshape(G, CHUNK, D).transpose(1, 0, 2)
                            for (n, h) in sel
                        ]
                    ),
                    np.ones((PER_CORE, CHUNK, G, 1), np.float32),
                ],
                axis=-1,
            ).astype(bf)
        )
        in_maps.append({"qt": qt, "kt": kt, "vn": vn, "mask": maskarr})
    return in_maps


def kernel(q: np.ndarray, k: np.ndarray, v: np.ndarray) -> np.ndarray:
    _install_bir_patch()
    from concourse.bass_utils import run_bass_kernel_spmd

    if "nc" not in _cached:
        _cached["nc"] = _build_bass()
    nc = _cached["nc"]

    in_maps = _prep_inputs(q, k, v)
    try:
        res = run_bass_kernel_spmd(nc, in_maps, core_ids=list(range(N_CORES)))
    except ModuleNotFoundError:
        # BASS_TRACE=1 with no axon NTFF hook in the container: retry untraced
        os.environ["BASS_NEVER_TRACE"] = "1"
        res = run_bass_kernel_spmd(nc, in_maps, core_ids=list(range(N_CORES)))
    _cached["last_result"] = res

    out = np.empty((B, L, H, D), np.float32)
    for c in range(N_CORES):
        sel = PAIRS[c * PER_CORE : (c + 1) * PER_CORE]
        for i, (n, h) in enumerate(sel):
            # on[i]: [CHUNK(c), G, D] -> [L, D]
            out[n, :, h, :] = (
                res.results[c]["on"][i]
                .astype(np.float32)
                .transpose(1, 0, 2)
                .reshape(L, D)
            )
    return out


# revision 50
# speedup vs baseline: 1.0179x; 1.0179x over previous
"""Causal linear attention (elu+1 feature map) Trainium2 Bass kernel.

Full inputs q,k,v: [4, 2048, 12, 64] fp32 -> out [4, 2048, 12, 64] fp32.
Sharding: 48 (batch, head) pairs, 6 per core across 8 NeuronCores.

bf16 end-to-end on device (inputs cast on host, output cast back):
matmuls run at 1 cycle/row, DMA bytes halve, DVE gets 2x/4x modes.

Per (n,h) pair, per 128-chunk g (exact algebraic regrouping):
  qf = phi(q), kf = phi(k), phi(x) = exp(min(x,0)) + max(x,0)
  state S_g = sum_{g'<g} kfc_{g'}^T @ [v | 1]   (PSUM kv outer products +
              one segmented DVE scan straight off PSUM)
  scoresT[s,c] = sum_d kfT[d,s] qfT[d,c], masked to s<=c
  num' = qfc @ S_{g-1} + scoresT-contract @ [v | 1]  -> [128, 65]
  out = num'[:, :64] * (1 / num'[:, 64])  (DVE reciprocal off the PSUM z
        column + one broadcast multiply per 4-chunk group)

Pairs are processed two at a time stacked on partition halves. The
kn layout ([s, g, d] with chunk position on partitions, needed as the
kv-matmul stationary operand) is produced from the phi'd kT tile by a
single XBAR DMA transpose per pair-pair: transposing [128(pair,d),
2048(g,s)] into [128(s), 16(g), 128(pair,d)] transposes each 128x128
chunk in place, so both pairs' kn tiles come out stacked correctly.
"""

import json
import os

import numpy as np

# ---------------------------------------------------------------------------
# Workaround for walrus "Too many sync wait commands": cap waits per
# instruction at 1, hoisting overflow onto same-engine NoOps inserted
# immediately before (engines run their stream in order, so consecutive
# waits AND together identically).
# ---------------------------------------------------------------------------
_wsplit_counter = [0]


def _split_instruction_waits(inst):
    si = inst.get("sync_info")
    if not si:
        return []
    waits = si.get("on_wait") or []
    if len(waits) <= 1:
        return []
    si["on_wait"] = waits[-1:]
    nops = []
    for w in waits[:-1]:
        _wsplit_counter[0] += 1
        nops.append(
            {
                "debug": inst.get("debug", 0),
                "engine": inst["engine"],
                "ins": [],
                "name": f"I-wsplit-{_wsplit_counter[0]}",
                "opcode": "NoOp",
                "outs": [],
                "sync_info": {"on_update": [], "on_wait": [w]},
            }
        )
    return nops


def _fix_module_json(raw: bytes) -> bytes:
    m = json.loads(raw)
    changed = False
    for f in m.get("functions", []):
        for b in f.get("blocks", []):
            out = []
            for inst in b.get("instructions", []):
                nops = _split_instruction_waits(inst)
                if nops:
                    changed = True
                    out.extend(nops)
                out.append(inst)
            b["instructions"] = out
    return json.dumps(m).encode() if changed else raw


_patch_installed = [False]


def _install_bir_patch():
    if _patch_installed[0]:
        return
    _patch_installed[0] = True
    import concourse.bass as _bass

    _orig = _bass.Bass.to_json_bytes

    def _patched(self):
        return _fix_module_json(_orig(self))

    _bass.Bass.to_json_bytes = _patched


# ---------------------------------------------------------------------------
# Problem constants (hardcoded per contest contract)
# ---------------------------------------------------------------------------
B, L, H, D = 4, 2048, 12, 64
CHUNK = 128
G = L // CHUNK  # 16
N_CORES = 8
PAIRS = [(n, h) for n in range(B) for h in range(H)]  # 48
PER_CORE = len(PAIRS) // N_CORES  # 6
NPP = PER_CORE // 2  # pair-pairs per core


DEFAULT_CFG = {
    "tpos": "after_q",   # transpose emission: after_k | after_q | mixed
    "qphi": "split",     # pool_mf | pool_il | dve | split (Pool+DVE)
    "kphi": "split",     # dve | split
    "sc_route": "ppppappp",  # per group: d=dve, a=actdve, p=actpool
    "nm_route": "dddddddd",  # per group: d=dve, a=act-chunks
    "bufs": (3, 3, 3),
    "loads": "half",
    "warmup": 0,
}


def _build_bass(reps: int = 1, cfg: dict | None = None):
    cfg = {**DEFAULT_CFG, **(cfg or {})}
    import concourse.bass as bass
    import concourse.tile as tile
    import concourse.mybir as mybir

    fp32 = mybir.dt.float32
    bf16 = mybir.dt.bfloat16
    AF = mybir.ActivationFunctionType
    ALU = mybir.AluOpType

    nc = bass.Bass()
    qt = nc.dram_tensor("qt", [NPP, 2 * D, L], bf16, kind="ExternalInput")
    kt = nc.dram_tensor("kt", [NPP, 2 * D, L], bf16, kind="ExternalInput")
    # v with the ones column baked in host-side: contiguous 2080B rows
    vn = nc.dram_tensor("vn", [PER_CORE, CHUNK, G, D + 1], bf16, kind="ExternalInput")
    mask = nc.dram_tensor("mask", [CHUNK, CHUNK], bf16, kind="ExternalInput")
    on = nc.dram_tensor("on", [PER_CORE, CHUNK, G, D], bf16, kind="ExternalOutput")

    SCB = 4  # chunks per PSUM bank batch

    with tile.TileContext(nc) as tc:
        with (
            tc.tile_pool(name="singles", bufs=1) as singles,
            tc.tile_pool(name="ins", bufs=cfg["bufs"][0]) as ins,
            tc.tile_pool(name="work", bufs=cfg["bufs"][1]) as work,
            tc.tile_pool(name="outs", bufs=cfg["bufs"][2]) as outs,
            tc.tile_pool(name="ps_sc", bufs=3, space="PSUM") as ps_sc,
            tc.tile_pool(name="ps_num", bufs=2, space="PSUM") as ps_num,
            tc.tile_pool(name="ps_kv", bufs=1, space="PSUM") as ps_kv_pool,
            tc.tile_pool(name="ps_kz", bufs=1, space="PSUM") as ps_kz_pool,
        ):
            maskbuf = singles.tile([CHUNK, SCB, CHUNK], bf16)
            segreset = singles.tile([CHUNK, D + 1, G], bf16)

            def emit_singles():
                # on Pool so DVE is free for phi from the first microsecond
                nc.sync.dma_start(out=maskbuf[:, 0, :], in_=mask[:])
                for i in range(1, SCB):
                    nc.gpsimd.tensor_copy(out=maskbuf[:, i, :], in_=maskbuf[:, 0, :])
                # segmented-scan reset pattern (g innermost): 0 at g==0, 1 else
                nc.gpsimd.memset(segreset[:], 1.0)
                nc.gpsimd.memset(segreset[:, :, 0:1], 0.0)

            # pp-level software pipeline: emission order per engine is
            # program order, so phi for pp j+1 is emitted BEFORE the
            # scores/num pipeline of pp j (DVE/Pool/ACT start it while PE
            # still works on pp j), and out DMAs are emitted after the next
            # pp's input loads so SP's in-order issue never starves.
            pending_outs = []

            def emit_loads(j):
                hp = tc.high_priority()
                hp.__enter__()
                qT2f = ins.tile([2 * D, L], bf16, tag="qT2f")
                kT2f = ins.tile([2 * D, L], bf16, tag="kT2f")
                vt2 = ins.tile([CHUNK, 2, G, D + 1], bf16, tag="vt2")
                if cfg["loads"] == "half":
                    for h in (0, 1):
                        hs = slice(h * (L // 2), (h + 1) * (L // 2))
                        nc.sync.dma_start(out=kT2f[:, hs], in_=kt[j, :, hs])
                        nc.sync.dma_start(out=qT2f[:, hs], in_=qt[j, :, hs])
                else:
                    nc.sync.dma_start(out=kT2f[:], in_=kt[j])
                    nc.sync.dma_start(out=qT2f[:], in_=qt[j])
                for s in (0, 1):
                    nc.sync.dma_start(out=vt2[:, s, :, :], in_=vn[2 * j + s])
                hp.__exit__(None, None, None)
                return qT2f, kT2f, vt2

            def flush_outs():
                for out_ap, in_ap in pending_outs:
                    nc.sync.dma_start(out=out_ap, in_=in_ap)
                pending_outs.clear()

            def emit_phi(j, tiles, tpos=None):
                qT2f, kT2f, vt2 = tiles
                hp = tc.high_priority()
                hp.__enter__()
                qT2 = work.tile([2 * D, L], bf16, tag="qT2")
                kT2 = work.tile([2 * D, L], bf16, tag="kT2")
                mx = work.tile([2 * D, L], bf16, tag="mx")
                mxq = work.tile([2 * D, L], bf16, tag="mxq")
                kn2b = work.tile([CHUNK, G, 2 * D], bf16, tag="kn2b")
                kmax = {0: nc.vector, 1: nc.vector}
                if cfg.get("kphi") == "split":
                    kmax = {0: nc.gpsimd, 1: nc.vector}
                for h in (0, 1):
                    hs = slice(h * (L // 2), (h + 1) * (L // 2))
                    nc.vector.tensor_scalar_min(
                        out=kT2[:, hs], in0=kT2f[:, hs], scalar1=0.0
                    )
                    nc.scalar.activation(out=kT2[:, hs], in_=kT2[:, hs], func=AF.Exp)
                    kmax[h].tensor_scalar_max(
                        out=mx[:, hs], in0=kT2f[:, hs], scalar1=0.0
                    )
                    nc.vector.tensor_tensor(
                        out=kT2[:, hs], in0=kT2[:, hs], in1=mx[:, hs], op=ALU.add
                    )
                tpos = tpos or cfg["tpos"]
                if tpos == "after_k":
                    nc.scalar.dma_start(out=kn2b[:], in_=kT2[:], transpose=True)
                qeng = nc.vector if cfg["qphi"] == "dve" else nc.gpsimd
                if cfg["qphi"] in ("pool_mf", "split"):
                    qmin = {0: qeng, 1: qeng}
                    qmax = {0: qeng, 1: qeng}
                    if cfg["qphi"] == "split":
                        qmin = {0: nc.gpsimd, 1: nc.vector}
                        qmax = {0: nc.vector, 1: nc.gpsimd}
                    for h in (0, 1):
                        hs = slice(h * (L // 2), (h + 1) * (L // 2))
                        qmin[h].tensor_scalar_min(
                            out=qT2[:, hs], in0=qT2f[:, hs], scalar1=0.0
                        )
                        nc.scalar.activation(
                            out=qT2[:, hs], in_=qT2[:, hs], func=AF.Exp
                        )
                    for h in (0, 1):
                        hs = slice(h * (L // 2), (h + 1) * (L // 2))
                        qmax[h].tensor_scalar_max(
                            out=mxq[:, hs], in0=qT2f[:, hs], scalar1=0.0
                        )
                        nc.vector.tensor_tensor(
                            out=qT2[:, hs], in0=qT2[:, hs], in1=mxq[:, hs],
                            op=ALU.add,
                        )
                else:
                    for h in (0, 1):
                        hs = slice(h * (L // 2), (h + 1) * (L // 2))
                        qeng.tensor_scalar_min(
                            out=qT2[:, hs], in0=qT2f[:, hs], scalar1=0.0
                        )
                        nc.scalar.activation(
                            out=qT2[:, hs], in_=qT2[:, hs], func=AF.Exp
                        )
                        qeng.tensor_scalar_max(
                            out=mxq[:, hs], in0=qT2f[:, hs], scalar1=0.0
                        )
                        nc.vector.tensor_tensor(
                            out=qT2[:, hs], in0=qT2[:, hs], in1=mxq[:, hs],
                            op=ALU.add,
                        )
                if tpos == "after_q":
                    if cfg.get("tsplit"):
                        for h in (0, 1):
                            hs = slice(h * (L // 2), (h + 1) * (L // 2))
                            nc.scalar.dma_start(
                                out=kn2b[:, h * (G // 2) : (h + 1) * (G // 2), :],
                                in_=kT2[:, hs],
                                transpose=True,
                            )
                    else:
                        nc.scalar.dma_start(out=kn2b[:], in_=kT2[:], transpose=True)
                hp.__exit__(None, None, None)
                return qT2, kT2, vt2, kn2b

            def emit_state(j, phi_tiles):
                qT2, kT2, vt2, kn2b = phi_tiles
                po_ = cfg.get("state_prio", None)
                hp = tc.high_priority(offset=po_) if po_ != "off" else None
                if hp: hp.__enter__()
                # per-chunk kv outer products, both pairs stacked; z column
                # in its own small PSUM tile so kv fits 2 banks
                kv_ps = ps_kv_pool.tile([CHUNK, G, D], fp32, tag="kv")
                kz_ps = ps_kz_pool.tile([CHUNK, G], fp32, tag="kz")
                for s in (0, 1):
                    po = D * s
                    for g in range(G):
                        nc.tensor.matmul(
                            kv_ps[po : po + D, g, :],
                            kn2b[:, g, po : po + D],
                            vt2[:, s, g, 0:D],
                            start=True, stop=True, skip_group_check=True,
                        )
                        nc.tensor.matmul(
                            kz_ps[po : po + D, g : g + 1],
                            kn2b[:, g, po : po + D],
                            vt2[:, s, g, D : D + 1],
                            start=True, stop=True, skip_group_check=True,
                        )
                kv_all = work.tile([CHUNK, D + 1, G], bf16, tag="kv_all")
                if cfg.get("kvcopy") == "split":
                    nc.scalar.copy(
                        out=kv_all[:, 0 : D // 2, :],
                        in_=kv_ps[:, :, 0 : D // 2].rearrange("p g m -> p m g"),
                    )
                    nc.vector.tensor_copy(
                        out=kv_all[:, D // 2 : D, :],
                        in_=kv_ps[:, :, D // 2 : D].rearrange("p g m -> p m g"),
                    )
                else:
                    nc.scalar.copy(
                        out=kv_all[:, 0:D, :],
                        in_=kv_ps[:].rearrange("p g m -> p m g"),
                    )
                nc.scalar.copy(out=kv_all[:, D, :], in_=kz_ps[:])
                kv_scan = work.tile([CHUNK, D + 1, G], bf16, tag="kv_scan")
                nc.vector.tensor_tensor_scan(
                    out=kv_scan[:].rearrange("p m g -> p (m g)"),
                    data0=segreset[:].rearrange("p m g -> p (m g)"),
                    data1=kv_all[:].rearrange("p m g -> p (m g)"),
                    initial=0.0,
                    op0=ALU.mult,
                    op1=ALU.add,
                )
                if hp: hp.__exit__(None, None, None)
                return qT2, kT2, vt2, kv_scan

            _scr = cfg["sc_route"] * (24 // len(cfg["sc_route"]) + 1)
            _nmr = cfg["nm_route"] * (24 // len(cfg["nm_route"]) + 1)
            SC_ROUTE = [
                {"d": "dve", "a": "actdve", "p": "actpool", "v": "dvepool"}[c]
                for c in _scr[:24]
            ]
            NM_ROUTE = [{"d": "dve", "a": "act"}[c] for c in _nmr[:24]]

            def make_group_emitters(j, st, goff):
                qT2, kT2, vt2, kv_scan = st
                out_stage = outs.tile([CHUNK, 2, G, D], bf16, tag="out_stage")
                groups = [(gb, s) for s in (0, 1) for gb in range(G // SCB)]
                state = {}

                def emit_sc(t):
                    gb, s = groups[t]
                    po = D * s
                    sc_ps = ps_sc.tile([CHUNK, SCB, CHUNK], fp32, tag="sc")
                    for i in range(SCB):
                        g = gb * SCB + i
                        cs = slice(g * CHUNK, (g + 1) * CHUNK)
                        nc.tensor.matmul(
                            sc_ps[:, i, :],
                            kT2[po : po + D, cs],
                            qT2[po : po + D, cs],
                            start=True, stop=True, skip_group_check=True,
                        )
                    state[t] = [sc_ps, None]

                def emit_evac_num(t):
                    gb, s = groups[t]
                    po = D * s
                    sc_ps = state[t][0]
                    sc_sb = work.tile([CHUNK, SCB, CHUNK], bf16, tag="sc_sb", bufs=10)
                    route = SC_ROUTE[goff + t]
                    if route == "dve":
                        nc.vector.tensor_tensor(
                            out=sc_sb[:], in0=sc_ps[:], in1=maskbuf[:], op=ALU.mult
                        )
                    else:
                        sc_raw = work.tile([CHUNK, SCB, CHUNK], bf16, tag="sc_raw", bufs=6)
                        if route == "dvepool":
                            nc.vector.tensor_copy(out=sc_raw[:], in_=sc_ps[:])
                        else:
                            nc.scalar.copy(out=sc_raw[:], in_=sc_ps[:])
                        if route == "actdve":
                            nc.vector.tensor_tensor(
                                out=sc_sb[:], in0=sc_raw[:], in1=maskbuf[:],
                                op=ALU.mult,
                            )
                        else:
                            # causal mask via index predicate: keep where
                            # c - s >= 0 (iota = -partition + col)
                            nc.gpsimd.affine_select(
                                out=sc_sb[:],
                                in_=sc_raw[:],
                                pattern=[[0, SCB], [1, CHUNK]],
                                base=0,
                                channel_multiplier=-1,
                                compare_op=ALU.is_ge,
                                fill=0.0,
                            )
                    num_ps = ps_num.tile([CHUNK, SCB, D + 1], fp32, tag="num")
                    for i in range(SCB):
                        g = gb * SCB + i
                        cs = slice(g * CHUNK, (g + 1) * CHUNK)
                        if g > 0:
                            nc.tensor.matmul(
                                num_ps[:, i, :],
                                qT2[po : po + D, cs],
                                kv_scan[po : po + D, :, g - 1],
                                start=True, stop=False, skip_group_check=True,
                            )
                        nc.tensor.matmul(
                            num_ps[:, i, :],
                            sc_sb[:, i, :],
                            vt2[:, s, g, :],
                            start=(g == 0), stop=True, skip_group_check=True,
                        )
                    state[t][1] = num_ps

                def emit_norm(t):
                    gb, s = groups[t]
                    num_ps = state.pop(t)[1]
                    r4 = work.tile([CHUNK, SCB], fp32, tag="r4", bufs=6)
                    nc.vector.reciprocal(out=r4[:], in_=num_ps[:, :, D])
                    if NM_ROUTE[goff + t] == "dve":
                        nc.vector.tensor_tensor(
                            out=out_stage[:, s, gb * SCB : (gb + 1) * SCB, :],
                            in0=num_ps[:, :, 0:D],
                            in1=r4[:].broadcast_to([CHUNK, SCB, D]),
                            op=ALU.mult,
                        )
                    else:
                        for i in range(SCB):
                            g = gb * SCB + i
                            nc.scalar.mul(
                                out=out_stage[:, s, g, :],
                                in_=num_ps[:, i, 0:D],
                                mul=r4[:, i : i + 1],
                            )

                def finish(t):
                    og = cfg.get("outgran", 1)
                    if (t + 1) % og:
                        return
                    s_ = t // 4
                    lo = (t - og + 1) % 4
                    gsl = slice(lo * SCB, (lo + og) * SCB)
                    pending_outs.append(
                        (on[2 * j + s_][:, gsl, :], out_stage[:, s_, gsl, :])
                    )

                return emit_sc, emit_evac_num, emit_norm, finish

            # -- merged cross-pp pipeline: one continuous stream of groups
            # over all pair-pairs; sc matmuls run 2 groups ahead of the
            # evacuation, normalize trails one more. Next pp's phi/state
            # and loads are interleaved at fixed slots.
            pps = [jj for _ in range(reps) for jj in range(NPP)]
            NG = 2 * (G // SCB)  # groups per pp
            npp_total = len(pps)
            raw = {}
            phi = {}
            emitters = {}

            if cfg["warmup"]:
                with tc.high_priority():
                    wtile = singles.tile([CHUNK, CHUNK], bf16, name="warm_in")
                    nc.gpsimd.memset(wtile[:], 0.0)
                    warm_ps = ps_sc.tile([CHUNK, SCB, CHUNK], fp32, tag="sc")
                    for i in range(cfg["warmup"]):
                        nc.tensor.matmul(
                            warm_ps[:, i % SCB, :], wtile[:], wtile[:],
                            start=True, stop=True, skip_group_check=True,
                        )
            raw[0] = emit_loads(pps[0])
            emit_singles()
            tp0 = "after_k" if cfg["tpos"] == "mixed" else cfg["tpos"]
            tps = "after_q" if cfg["tpos"] == "mixed" else cfg["tpos"]
            phi[0] = emit_phi(pps[0], raw.pop(0), tpos=tp0)
            emitters[0] = make_group_emitters(
                pps[0], emit_state(pps[0], phi.pop(0)), 0
            )
            if npp_total > 1:
                raw[1] = emit_loads(pps[1])

            total_slots = npp_total * NG + 3
            for u in range(total_slots):
                idx, t = divmod(u, NG)
                if t == 1 and idx + 1 < npp_total:
                    phi[idx + 1] = emit_phi(pps[idx + 1], raw.pop(idx + 1), tpos=tps)
                if t == 1 and idx + 2 < npp_total:
                    raw[idx + 2] = emit_loads(pps[idx + 2])
                    flush_outs()
                if t == 5 and idx + 1 < npp_total:
                    emitters[idx + 1] = make_group_emitters(
                        pps[idx + 1],
                        emit_state(pps[idx + 1], phi.pop(idx + 1)),
                        (idx + 1) * NG,
                    )
                if u < npp_total * NG:
                    emitters[idx][0](t)  # sc
                if 2 <= u < npp_total * NG + 2:
                    i2, t2 = divmod(u - 2, NG)
                    emitters[i2][1](t2)  # evac + num
                if 3 <= u < npp_total * NG + 3:
                    i3, t3 = divmod(u - 3, NG)
                    emitters[i3][2](t3)  # norm
                    emitters[i3][3](t3)  # queue completed out slice
                    if t3 == NG - 1:
                        del emitters[i3]
            flush_outs()
    return nc


_cached = {}


def _prep_inputs(q, k, v):
    import ml_dtypes

    bf = ml_dtypes.bfloat16
    maskarr = np.ascontiguousarray(
        np.tril(np.ones((CHUNK, CHUNK), np.float32)).T
    ).astype(bf)  # [s, c] : 1 if s<=c
    in_maps = []
    for c in range(N_CORES):
        sel = PAIRS[c * PER_CORE : (c + 1) * PER_CORE]
        qt = np.ascontiguousarray(
            np.stack([q[n, :, h, :].T for (n, h) in sel]).astype(bf)
        ).reshape(NPP, 2 * D, L)
        kt = np.ascontiguousarray(
            np.stack([k[n, :, h, :].T for (n, h) in sel]).astype(bf)
        ).reshape(NPP, 2 * D, L)
        vn = np.ascontiguousarray(
            np.concatenate(
                [
                    np.stack(
                        [
                            v[n, :, h, :].reshape(G, CHUNK, D).transpose(1, 0, 2)
                            for (n, h) in sel
                        ]
                    ),
                    np.ones((PER_CORE, CHUNK, G, 1), np.float32),
                ],
                axis=-1,
            ).astype(bf)
        )
        in_maps.append({"qt": qt, "kt": kt, "vn": vn, "mask": maskarr})
    return in_maps


def kernel(q: np.ndarray, k: np.ndarray, v: np.ndarray) -> np.ndarray:
    _install_bir_patch()
    from concourse.bass_utils import run_bass_kernel_spmd

    if "nc" not in _cached:
        _cached["nc"] = _build_bass()
    nc = _cached["nc"]

    in_maps = _prep_inputs(q, k, v)
    try:
        res = run_bass_kernel_spmd(nc, in_maps, core_ids=list(range(N_CORES)))
    except ModuleNotFoundError:
        # BASS_TRACE=1 with no axon NTFF hook in the container: retry untraced
        os.environ["BASS_NEVER_TRACE"] = "1"
        res = run_bass_kernel_spmd(nc, in_maps, core_ids=list(range(N_CORES)))
    _cached["last_result"] = res

    out = np.empty((B, L, H, D), np.float32)
    for c in range(N_CORES):
        sel = PAIRS[c * PER_CORE : (c + 1) * PER_CORE]
        for i, (n, h) in enumerate(sel):
            # on[i]: [CHUNK(c), G, D] -> [L, D]
            out[n, :, h, :] = (
                res.results[c]["on"][i]
                .astype(np.float32)
                .transpose(1, 0, 2)
                .reshape(L, D)
            )
    return out
